# revision 7
# baseline (speedup 1.0000x reference)
"""InfiniAttention TRN2 kernel: 8-core data-parallel over (B*S) rows.

Shapes (hardcoded): B=4, S=4096, HID=2048, H=16, NB=4, D=128.
Each core processes 2048 rows. Per-head fused pipeline:
  qT/kT/vT projections (feature-major) -> elu+1 -> retrieval from 4 banks
  -> delta-rule update partials -> combined^T spilled to DRAM scratch
  -> streaming out-projection.
Cross-core reductions (mem update / norm sums) done on host at unshard time.
"""

import os
import sys

import numpy as np
import ml_dtypes

sys.path.insert(0, "/opt/trn_rl_repo")

import concourse.bass as bass
import concourse.bacc as bacc
import concourse.tile as tile
import concourse.mybir as mybir
from concourse.bass_utils import run_bass_kernel_spmd

B, S, HID = 4, 4096, 2048
H, NB, D = 16, 4, 128
P = 128          # partitions
KC = 16          # k chunks (HID / 128)
NCORES = 8
M = (B * S) // NCORES  # rows per core = 2048
MT = M // P      # m tiles per core = 16
MB = 4           # moving blocks of 512 per projection psum
EPS = 1e-6

f32 = mybir.dt.float32
bf16 = mybir.dt.bfloat16
AF = mybir.ActivationFunctionType
ALU = mybir.AluOpType

bf = ml_dtypes.bfloat16

LAST_RESULTS = None  # stashed BassKernelResults for test harness
LAST_IN_MAPS = None  # stashed per-core inputs for rerun timing


def _build_bass():
    nc = bacc.Bacc(
        "TRN2",
        target_bir_lowering=False,
        debug=False,
        enable_asserts=False,
        num_devices=NCORES,
    )

    # ---- DRAM I/O -------------------------------------------------------
    xt_d = nc.dram_tensor("xt", [KC, P, M], bf16, kind="ExternalInput").ap()
    wq_d = nc.dram_tensor("wq", [H, P, KC * P], bf16, kind="ExternalInput").ap()
    wk_d = nc.dram_tensor("wk", [H, P, KC * P], bf16, kind="ExternalInput").ap()
    wv_d = nc.dram_tensor("wv", [H, P, KC * P], bf16, kind="ExternalInput").ap()
    wo_d = nc.dram_tensor("wo", [H, P, HID], bf16, kind="ExternalInput").ap()
    memp_d = nc.dram_tensor("memp", [H, P, NB * D], bf16, kind="ExternalInput").ap()
    rkp_d = nc.dram_tensor("rkp", [H, P, 132], bf16, kind="ExternalInput").ap()
    nq_d = nc.dram_tensor("nq", [H, P, NB], bf16, kind="ExternalInput").ap()
    id_d = nc.dram_tensor("ident", [P, P], bf16, kind="ExternalInput").ap()

    outp_d = nc.dram_tensor("outp", [HID, M], f32, kind="ExternalOutput").ap()
    upd_d = nc.dram_tensor("upd", [H, D, D], f32, kind="ExternalOutput").ap()
    ksum_d = nc.dram_tensor("ksum", [P, H], f32, kind="ExternalOutput").ap()

    with tile.TileContext(nc) as tc:
        with (
            tc.tile_pool(name="dramp", bufs=1, space="DRAM") as dramp,
            tc.tile_pool(name="const", bufs=1) as constp,
            tc.tile_pool(name="ksums", bufs=1) as ksp,
        ):
            combt_dram = dramp.tile([P, H * M], bf16)  # combined^T spill
            ident = constp.tile([P, P], bf16)
            nc.sync.dma_start(ident[:], id_d[:, :])
            ksum_sb = ksp.tile([P, H], f32)

            with (
                tc.tile_pool(name="pp", bufs=2, space="PSUM") as pp,   # 2 banks
                tc.tile_pool(name="ap", bufs=3, space="PSUM") as apl,  # 3 banks
                tc.tile_pool(name="sp", bufs=2, space="PSUM") as sp,   # 2 banks
                tc.tile_pool(name="up", bufs=1, space="PSUM") as up,   # 1 bank
                tc.tile_pool(name="xt", bufs=1) as xtp,
                tc.tile_pool(name="wqkv", bufs=2) as wp,
                tc.tile_pool(name="mems", bufs=1) as memsp,
                tc.tile_pool(name="sig", bufs=2) as sigp,
                tc.tile_pool(name="eluT", bufs=3) as elup,
                tc.tile_pool(name="ctstage", bufs=2) as ctsp,
                tc.tile_pool(name="smrm", bufs=3) as smp,
                tc.tile_pool(name="updsb", bufs=2) as updsbp,
                tc.tile_pool(name="recp", bufs=3) as recp,
            ):
                # resident inputs
                xt = xtp.tile([P, KC * M], bf16)
                for kc in range(KC):
                    nc.sync.dma_start(xt[:, kc * M:(kc + 1) * M], xt_d[kc])
                memp = memsp.tile([P, H * NB * D], bf16, tag="memp")
                rkp = memsp.tile([P, H * 132], bf16, tag="rkp")
                nqt = memsp.tile([P, H * NB], bf16, tag="nq")
                for h in range(H):
                    nc.sync.dma_start(memp[:, h * NB * D:(h + 1) * NB * D], memp_d[h])
                    nc.sync.dma_start(rkp[:, h * 132:(h + 1) * 132], rkp_d[h])
                    nc.sync.dma_start(nqt[:, h * NB:(h + 1) * NB], nq_d[h])

                for h in range(H):
                    # -- load this head's weight column blocks --
                    wqh = wp.tile([P, KC * P], bf16, tag="wq")
                    wkh = wp.tile([P, KC * P], bf16, tag="wk")
                    wvh = wp.tile([P, KC * P], bf16, tag="wv")
                    nc.sync.dma_start(wqh[:], wq_d[h])
                    nc.sync.dma_start(wkh[:], wk_d[h])
                    nc.sync.dma_start(wvh[:], wv_d[h])

                    # -- projections: feature-major [128, M] per head --
                    sq = sigp.tile([P, M], bf16, tag="sq")
                    sk = sigp.tile([P, M], bf16, tag="sk")
                    vT = sigp.tile([P, M], bf16, tag="vT")
                    for name, wh, dst in (("q", wqh, sq), ("k", wkh, sk), ("v", wvh, vT)):
                        for mb in range(MB):
                            ps = pp.tile([P, 512], f32, tag="proj")
                            for kc in range(KC):
                                nc.tensor.matmul(
                                    ps[:],
                                    wh[:, kc * P:(kc + 1) * P],
                                    xt[:, kc * M + mb * 512: kc * M + (mb + 1) * 512],
                                    start=(kc == 0),
                                    stop=(kc == KC - 1),
                                )
                            dslice = dst[:, mb * 512:(mb + 1) * 512]
                            if name == "v":
                                nc.scalar.copy(dslice, ps[:])
                            else:
                                # elu(x)+1 = relu(x) + min(exp(x), 1)
                                ex = elup.tile([P, 512], f32, tag="ex")
                                rl = elup.tile([P, 512], f32, tag="rl")
                                nc.scalar.activation(ex[:], ps[:], AF.Exp)
                                nc.scalar.activation(rl[:], ps[:], AF.Relu)
                                nc.vector.scalar_tensor_tensor(
                                    dslice, ex[:], 1.0, rl[:], ALU.min, ALU.add
                                )

                    # ksum[:, h] = sum_m sigma_k
                    nc.vector.reduce_sum(
                        ksum_sb[:, h:h + 1], sk[:], axis=mybir.AxisListType.X
                    )

                    ct = ctsp.tile([P, M], bf16, tag="ct")  # combined^T for head h
                    updps = up.tile([P, D], f32, tag="upd")
                    for mt in range(MT):
                        mts = slice(mt * P, (mt + 1) * P)
                        # retrieval: a_all = sigma_q_mt^T @ [mem banks]
                        aall = apl.tile([P, 512], f32, tag="a")
                        nc.tensor.matmul(
                            aall[:], sq[:, mts], memp[:, h * NB * D:(h + 1) * NB * D],
                            start=True, stop=True,
                        )
                        smn = sp.tile([P, 192], f32, tag="smn")
                        nc.tensor.matmul(
                            smn[:, 140:140 + NB], sq[:, mts],
                            nqt[:, h * NB:(h + 1) * NB], start=True, stop=True,
                        )
                        # retr (128 cols) + knorm (col 128)
                        nc.tensor.matmul(
                            smn[:, 0:129], sk[:, mts],
                            rkp[:, h * 132: h * 132 + 129], start=True, stop=True,
                        )
                        # transposes of sigma_k, v for row-major use
                        trk = apl.tile([P, P], bf16, tag="a")
                        nc.tensor.transpose(trk[:], sk[:, mts], ident[:])
                        trv = apl.tile([P, P], bf16, tag="a")
                        nc.tensor.transpose(trv[:], vT[:, mts], ident[:])
                        skrm = smp.tile([P, P], bf16, tag="skrm")
                        vrm = smp.tile([P, P], bf16, tag="vrm")
                        nc.vector.tensor_copy(skrm[:], trk[:])
                        nc.vector.tensor_copy(vrm[:], trv[:])

                        # per-bank normalize+weight, accumulate combined (row-major)
                        rec = recp.tile([P, NB], f32, tag="rec")
                        nc.vector.tensor_scalar(
                            rec[:], smn[:, 140:140 + NB], EPS, None, ALU.max
                        )
                        nc.vector.reciprocal(rec[:], rec[:])
                        cs = smp.tile([P, P], bf16, tag="cs")
                        nc.vector.tensor_scalar_mul(cs[:], aall[:, 0:D], rec[:, 0:1])
                        for n in range(1, NB):
                            nc.vector.scalar_tensor_tensor(
                                cs[:], aall[:, n * D:(n + 1) * D], rec[:, n:n + 1],
                                cs[:], ALU.mult, ALU.add,
                            )
                        # delta_v = v - retr/knorm  (krecn = -1/max(knorm, eps))
                        krecn = recp.tile([P, 1], f32, tag="krecn")
                        nc.vector.tensor_scalar(
                            krecn[:], smn[:, 128:129], EPS, -1.0, ALU.max, ALU.mult
                        )
                        nc.vector.reciprocal(krecn[:], krecn[:])
                        dv = smp.tile([P, P], bf16, tag="dv")
                        nc.vector.scalar_tensor_tensor(
                            dv[:], smn[:, 0:D], krecn[:, 0:1], vrm[:],
                            ALU.mult, ALU.add,
                        )
                        # mem update partial accumulation over mt
                        nc.tensor.matmul(
                            updps[:], skrm[:], dv[:],
                            start=(mt == 0), stop=(mt == MT - 1),
                            skip_group_check=True,
                        )
                        # transpose combined slice into ct
                        trc = apl.tile([P, P], bf16, tag="a")
                        nc.tensor.transpose(trc[:], cs[:], ident[:])
                        nc.vector.tensor_copy(ct[:, mts], trc[:])

                    # spill combined^T, write upd partial
                    nc.sync.dma_start(combt_dram[:, h * M:(h + 1) * M], ct[:])
                    updsb = updsbp.tile([P, D], f32, tag="updsb")
                    nc.scalar.copy(updsb[:], updps[:])
                    nc.sync.dma_start(upd_d[h], updsb[:])

                nc.sync.dma_start(ksum_d[:, :], ksum_sb[:])

            # ---- phase F: out^T projection (Wo stationary: 256 LDW) ----
            with (
                tc.tile_pool(name="fp", bufs=8, space="PSUM") as fp,
                tc.tile_pool(name="ctf", bufs=1) as ctfp,
                tc.tile_pool(name="wof", bufs=1) as wofp,
                tc.tile_pool(name="outs", bufs=4) as outsp,
            ):
                ctf = ctfp.tile([P, H * M], bf16)
                nc.sync.dma_start(ctf[:], combt_dram[:])
                wof = wofp.tile([P, H * HID], bf16)
                for h in range(H):
                    nc.sync.dma_start(wof[:, h * HID:(h + 1) * HID], wo_d[h])
                for nt in range(KC):  # 16 output-feature tiles of 128
                    pss = [fp.tile([P, 512], f32, tag="fo", name=f"fo{nt}_{i}") for i in range(MB)]
                    for h in range(H):
                        lhs = wof[:, h * HID + nt * P: h * HID + (nt + 1) * P]
                        for mb in range(MB):
                            nc.tensor.matmul(
                                pss[mb][:], lhs,
                                ctf[:, h * M + mb * 512: h * M + (mb + 1) * 512],
                                start=(h == 0), stop=(h == H - 1),
                                skip_group_check=True,
                            )
                    for mb in range(MB):
                        osb = outsp.tile([P, 512], f32, tag="o")
                        nc.scalar.copy(osb[:], pss[mb][:])
                        nc.sync.dma_start(
                            outp_d[nt * P:(nt + 1) * P, mb * 512:(mb + 1) * 512],
                            osb[:],
                        )
    return nc


_NC_CACHE = None


def _get_nc():
    global _NC_CACHE
    if _NC_CACHE is None:
        _NC_CACHE = _build_bass()
        if not _NC_CACHE.is_finalized():
            _NC_CACHE.finalize()
    return _NC_CACHE


def _prep_inputs(hidden_states, Wq, Wk, Wv, Wo, bank_weights, memories, memory_norms):
    """Host-side shard + relayout. Returns list of per-core input dicts."""
    f = np.float32
    hs = np.asarray(hidden_states, f).reshape(B * S, HID)
    WqT = np.asarray(Wq, f).T
    WkT = np.asarray(Wk, f).T
    WvT = np.asarray(Wv, f).T
    WoT = np.asarray(Wo, f).T
    bw = np.asarray(bank_weights, f)
    mem = np.asarray(memories, f)
    mn = np.asarray(memory_norms, f)

    # softmax over banks (stable, like jax)
    e = np.exp(bw - bw.max(axis=-1, keepdims=True))
    w_soft = e / e.sum(axis=-1, keepdims=True)          # [H, NB]
    active = (mn.sum(axis=(1, 2)) >= EPS).astype(f)     # [NB]

    def wtile(WT):  # [HID, HID] -> [H, P, KC*P]; w[h, p, kc*128+n] = WT[kc*128+p, h*128+n]
        return np.ascontiguousarray(
            WT.reshape(KC, P, H, P).transpose(2, 1, 0, 3).reshape(H, P, KC * P)
        ).astype(bf)

    wq = wtile(WqT)
    wk = wtile(WkT)
    wv = wtile(WvT)
    # wo[h, p, n] = WoT[h*128+p, n]  (head-row tiling of WoT)
    wo = np.ascontiguousarray(WoT.reshape(H, P, HID)).astype(bf)

    # memp[h, d, n*128+e] = memories[n,h,d,e] * w_soft[h,n] * active[n]
    memw = mem * (w_soft.T * active[:, None])[:, :, None, None]  # [NB,H,D,D]
    memp = np.ascontiguousarray(memw.transpose(1, 2, 0, 3).reshape(H, D, NB * D)).astype(bf)
    # rkp[h, d, 0:128]=mem0, [h, d, 128]=norm0
    rkp = np.zeros((H, D, 132), f)
    rkp[:, :, :D] = mem[0]
    rkp[:, :, D] = mn[0].transpose(0, 1)  # [H, D]
    rkp = rkp.astype(bf)
    # nq[h, d, n] = memory_norms[n, h, d]
    nq = np.ascontiguousarray(mn.transpose(1, 2, 0)).astype(bf)
    ident = np.eye(P, dtype=bf)

    shared = dict(wq=wq, wk=wk, wv=wv, wo=wo, memp=memp, rkp=rkp, nq=nq, ident=ident)
    in_maps = []
    for c in range(NCORES):
        shard = hs[c * M:(c + 1) * M]                    # [M, HID]
        xt = np.ascontiguousarray(shard.T.reshape(KC, P, M)).astype(bf)
        in_maps.append(dict(shared, xt=xt))
    return in_maps, mem, mn


def rerun():
    """Re-execute with cached inputs (for steady-state timing)."""
    import time
    nc = _get_nc()
    t0 = time.time()
    run_bass_kernel_spmd(nc, LAST_IN_MAPS, core_ids=list(range(NCORES)))
    return time.time() - t0


def kernel(hidden_states, Wq, Wk, Wv, Wo, bank_weights, memories, memory_norms):
    global LAST_RESULTS, LAST_IN_MAPS
    in_maps, mem, mn = _prep_inputs(
        hidden_states, Wq, Wk, Wv, Wo, bank_weights, memories, memory_norms
    )
    LAST_IN_MAPS = in_maps
    nc = _get_nc()
    res = run_bass_kernel_spmd(
        nc,
        in_maps,
        core_ids=list(range(NCORES)),
        trace=bool(int(os.environ.get("KERNEL_TRACE", "0"))),
    )
    LAST_RESULTS = res
    outs = res.results

    out = np.concatenate([outs[c]["outp"].T for c in range(NCORES)], axis=0)
    out = np.ascontiguousarray(out).reshape(B, S, HID)
    upd = np.sum([outs[c]["upd"] for c in range(NCORES)], axis=0)
    ksum = np.sum([outs[c]["ksum"] for c in range(NCORES)], axis=0)  # [P, H]
    new_mem0 = (mem[0] + upd / (B * S)).astype(np.float32)
    new_norm0 = (mn[0] + ksum.T / B).astype(np.float32)
    return out, new_mem0, new_norm0
